# revision 13
# baseline (speedup 1.0000x reference)
"""Trainium2 Bass kernel for nn_AdaptivePatchEmbedding.

Reference computes, over a [3,1024,1024] image:
  e0: 16x16 patches -> flatten -> @ Wb + b                    (8192 patches)
  e1: 32x32 patches -> bilinear-resize to 16x16 -> @ Wb + b   (4096 patches)
  e2: 64x64 patches -> bilinear-resize to 16x16 -> @ Wb + b   (2048 patches)
plus a ControlNet zero-init MLP branch on e1/e2 that is exactly zero for the
zero mlp weights (host numpy fallback keeps correctness otherwise).

Identities used:
  - 16x16/stride-16 conv == flatten + matmul with Wb = base_w.reshape(D,-1).T
  - bilinear 32->16 (half-pixel) == mean of each 2x2 block
  - bilinear 64->16 == mean of the 2x2 block at rows {4i+1,4i+2} x cols {4j+1,4j+2}

Data-parallel over patches across 8 cores (the sharding hint's "gathered
patch batches"): the host gathers + pre-averages all patches (0.25 pre-scale
for e1/e2, column order (c,r,j) = Wb row order) AND pre-transposes them into
k-major layout, so the device never transposes anything.

The GEMM runs in fp8-e4m3 DoubleRow mode (2 columns/cycle) with hi/lo error
compensation:  X = Xh + Xl (two fp8 splits),  W*64 = Wh + Wl.  Adjacent
k-tiles form the two DoubleRow planes, so per PSUM bank each job needs
9 DoubleRow matmuls covering  (Xh+Xl)@Wh + Xh@Wl  (the dropped Xl@Wl term
is ~1e-3 relative).  The host divides the result by 64 and adds base_b.

Per core: 14 jobs of 128 patches:
  direct DMA X^T hi/lo k-tiles [128,1536]fp8 in (job 0 on the Sync/HWDGE
  queue, jobs 1+ on the GpSimd/SWDGE queue, racing the weight stream)
  -> 18 DoubleRow matmuls (X^T tile-pairs stationary, f32 PSUM, 448/320
     bank split) -> Scalar/DVE copy-convert to bf16
  -> two half-row DMAs out (Sync queue).
Wh/Wl pair-chunks DMA'd in exact matmul-consumption order on the Scalar
queue.  A short PE warm-up (plain matmuls on a memset tile, no DMA
dependency) ramps the PE clock gate while the first input DMA is in
flight.
"""

import os
import sys

for _p in ("/opt/trn_rl_repo", "/root/.axon_site/_ro/trn_rl_repo"):
    if os.path.isdir(_p) and _p not in sys.path:
        sys.path.insert(0, _p)

import numpy as np
import ml_dtypes

BF16 = ml_dtypes.bfloat16
E4M3 = ml_dtypes.float8_e4m3

C = 3
H = W = 1024
D = 768
BASE = 16
N0, N1, N2 = 8192, 4096, 2048
NCORES = 8
P0, P1, P2 = N0 // NCORES, N1 // NCORES, N2 // NCORES  # 1024, 512, 256
G0, G1, G2 = P0 // 128, P1 // 128, P2 // 128  # 8, 4, 2 jobs of 128 patches
NJOBS = G0 + G1 + G2  # 14
NKT = 6
NWARM = 22
WS = 64.0  # W prescale into the fp8 normal range; host divides out by WS

_COMPILED = None


def _gather_host(image, coords0, coords1, coords2):
    """Gather + pre-average all patches into [N, 768] f32 (per level).

    Column order is (c, r, j) = Wb's row order; e1/e2 values are the 2x2
    block means (pre-scaled by 0.25) so every row embeds with the same Wb.
    """
    imgT = np.ascontiguousarray(image.transpose(1, 2, 0))  # [H, W, C] f32
    r16 = np.arange(16)
    j16 = np.arange(16)

    y, x = coords0[:, 0], coords0[:, 1]
    m = imgT[y[:, None, None] + r16[None, :, None],
             x[:, None, None] + j16[None, None, :]]  # [N, r, j, c]
    x0 = m.transpose(0, 3, 1, 2).reshape(-1, D)

    # row-pair sums [1023, 1024, 3]
    e1r = imgT[:-1] + imgT[1:]
    # col-pair sums at the two x-phases -> [2, 1023, 512, 3]
    e1rc = np.zeros((2, H - 1, W // 2, C), np.float32)
    e1rc[0] = e1r[:, 0::2] + e1r[:, 1::2]
    e1rc[1, :, :511] = e1r[:, 1:-1:2] + e1r[:, 2::2]

    y, x = coords1[:, 0], coords1[:, 1]
    p = x & 1
    x2 = (x - p) >> 1
    m = e1rc[p[:, None, None],
             y[:, None, None] + 2 * r16[None, :, None],
             x2[:, None, None] + j16[None, None, :]]  # [N, r, j, c]
    x1 = 0.25 * m.transpose(0, 3, 1, 2).reshape(-1, D)

    y, x = coords2[:, 0], coords2[:, 1]
    p = (x + 1) & 1
    x2 = (x + 1 - p) >> 1
    m = e1rc[p[:, None, None],
             (y + 1)[:, None, None] + 4 * r16[None, :, None],
             x2[:, None, None] + 2 * j16[None, None, :]]
    x2g = 0.25 * m.transpose(0, 3, 1, 2).reshape(-1, D)

    return x0.astype(np.float32), x1.astype(np.float32), x2g.astype(np.float32)


def _build_graph():
    import concourse.bass as bass
    import concourse.mybir as mybir
    from concourse import bacc
    import concourse.tile as tile

    nc = bacc.Bacc("TRN2", target_bir_lowering=False, debug=False)
    f32 = mybir.dt.float32
    bf16 = mybir.dt.bfloat16
    f8 = mybir.dt.float8e4
    DR = mybir.MatmulPerfMode.DoubleRow

    xin_d = nc.dram_tensor("xin", [128, NJOBS * 2 * D], f8, kind="ExternalInput")
    wh_d = nc.dram_tensor("wh", [128, NKT * D], f8, kind="ExternalInput")
    wl_d = nc.dram_tensor("wl", [128, NKT * D], f8, kind="ExternalInput")
    out_d = nc.dram_tensor("out", [NJOBS * 128, D], bf16, kind="ExternalOutput")

    with tile.TileContext(nc) as tc:
        with (
            tc.tile_pool(name="static", bufs=1) as st,
            tc.tile_pool(name="xp", bufs=6) as xp,
            tc.tile_pool(name="psA", bufs=3, space="PSUM") as psA,
            tc.tile_pool(name="psW", bufs=2, space="PSUM") as psW,
            tc.tile_pool(name="outp", bufs=6) as outp,
        ):
            def in_job(g, eng=None):
                xt = xp.tile([128, 2 * NKT * 128], f8, tag="xt")
                (eng or nc.gpsimd).dma_start(
                    xt[:], xin_d[:, g * 2 * D:(g + 1) * 2 * D])
                return xt

            # job 0 rides the HWDGE path (Sync) in parallel with the Pool
            # SWDGE stream that feeds jobs 1+
            pre = [in_job(0, nc.sync)] + [in_job(g) for g in range(1, 4)]

            # weight pair-chunks next (Scalar queue), in matmul consumption
            # order: (Wh pair0, Wl pair0, Wh pair1, ...) so the first jobs'
            # W-stream stalls are as fine-grained as possible
            wh_t = st.tile([128, NKT * D], f8, tag="wh")
            wl_t = st.tile([128, NKT * D], f8, tag="wl")
            for j in range(NKT // 2):
                s = 2 * j * D
                e = (2 * j + 2) * D
                nc.scalar.dma_start(wh_t[:, s:e], wh_d[:, s:e])
                nc.scalar.dma_start(wl_t[:, s:e], wl_d[:, s:e])

            ones = st.tile([128, 128], bf16, tag="ones")
            nc.vector.memset(ones[:], 0.25)
            # Scalar act-table preload off the critical path
            warm_s = st.tile([128, 8], bf16, tag="warm")
            nc.scalar.copy(warm_s[:], ones[:, 0:8])
            # PE clock-gate ramp during the first input DMA's flight
            for _ in range(NWARM):
                wps = psW.tile([128, 128], f32, tag="wp")
                nc.tensor.matmul(wps[:], ones[:], ones[:], start=True, stop=True)

            NS = 448  # Scalar-copied split; DVE takes the rest

            def embed_job(xt, g):
                acc0 = psA.tile([128, NS], f32, tag="acc0")
                acc1 = psA.tile([128, D - NS], f32, tag="acc1")
                for j in range(NKT // 2):
                    # DoubleRow planes = adjacent k-tiles (2j, 2j+1)
                    lhs_h = xt[:, 2 * j * 128:(2 * j + 2) * 128].rearrange(
                        "k (h p) -> k h p", h=2)
                    lhs_l = xt[:, NKT * 128 + 2 * j * 128:
                               NKT * 128 + (2 * j + 2) * 128].rearrange(
                        "k (h p) -> k h p", h=2)
                    whv = wh_t[:, 2 * j * D:(2 * j + 2) * D].rearrange(
                        "k (h n) -> k h n", h=2)
                    wlv = wl_t[:, 2 * j * D:(2 * j + 2) * D].rearrange(
                        "k (h n) -> k h n", h=2)
                    first = (j == 0)
                    last = (j == NKT // 2 - 1)
                    nc.tensor.matmul(acc0[:], lhs_h, whv[:, :, 0:NS],
                                     start=first, stop=False, perf_mode=DR)
                    nc.tensor.matmul(acc1[:], lhs_h, whv[:, :, NS:D],
                                     start=first, stop=False, perf_mode=DR)
                    nc.tensor.matmul(acc0[:], lhs_h, wlv[:, :, 0:NS],
                                     start=False, stop=False, perf_mode=DR)
                    nc.tensor.matmul(acc1[:], lhs_h, wlv[:, :, NS:D],
                                     start=False, stop=False, perf_mode=DR)
                    nc.tensor.matmul(acc0[:], lhs_l, whv[:, :, 0:NS],
                                     start=False, stop=last, perf_mode=DR)
                    nc.tensor.matmul(acc1[:], lhs_l, whv[:, :, NS:D],
                                     start=False, stop=last, perf_mode=DR)
                o_t = outp.tile([128, D], bf16, tag="o")
                nc.scalar.copy(o_t[:, 0:NS], acc0[:])
                nc.vector.tensor_copy(o_t[:, NS:D], acc1[:])
                nc.sync.dma_start(out_d[g * 128:(g + 1) * 128, 0:NS], o_t[:, 0:NS])
                nc.sync.dma_start(out_d[g * 128:(g + 1) * 128, NS:D], o_t[:, NS:D])

            for g in range(NJOBS):
                xt = pre[g] if g < 4 else in_job(g)
                embed_job(xt[:], g)

    nc.compile()
    return nc


def _get_compiled():
    global _COMPILED
    if _COMPILED is None:
        _COMPILED = _build_graph()
    return _COMPILED


def _mlp_correction(image, coords, g, agg_w, agg_b, mlp_w, mlp_b, base_w, base_b):
    """Host fallback: the zero-init-MLP branch, exact reference math."""
    Wb = base_w.reshape(D, -1).T
    ps = BASE * g
    n = coords.shape[0]
    patches = np.empty((n, C, ps, ps), np.float32)
    for k in range(n):
        y, x = int(coords[k, 0]), int(coords[k, 1])
        patches[k] = image[:, y:y + ps, x:x + ps]
    sub = patches.reshape(n, C, g, BASE, g, BASE).transpose(0, 2, 4, 1, 3, 5)
    sub_e = sub.reshape(n, g, g, C * BASE * BASE) @ Wb + base_b
    agg = np.einsum('nhwd,odhw->no', sub_e, agg_w) + agg_b
    return agg @ mlp_w.T + mlp_b


def _pretranspose(x):
    """[1792, 768] -> [128 k', NJOBS, 768] with [kt-major, patch] runs."""
    return x.reshape(NJOBS, 128, NKT, 128).transpose(3, 0, 2, 1).reshape(
        128, NJOBS, D)


def build_in_maps(image, coords0, coords1, coords2, base_w, base_b):
    coords0 = np.asarray(coords0).astype(np.int64)
    coords1 = np.asarray(coords1).astype(np.int64)
    coords2 = np.asarray(coords2).astype(np.int64)
    x0, x1, x2 = _gather_host(image, coords0, coords1, coords2)

    Wb = base_w.reshape(D, -1).T  # [768 k, 768 n]
    W64 = (Wb * WS).astype(np.float32)
    Wh = W64.astype(E4M3)
    Wl = (W64 - Wh.astype(np.float32)).astype(E4M3)

    def wtile(w):
        return np.ascontiguousarray(
            w.reshape(NKT, 128, D).transpose(1, 0, 2).reshape(128, NKT * D))

    wh_np, wl_np = wtile(Wh), wtile(Wl)

    in_maps = []
    for k in range(NCORES):
        xc = np.concatenate([
            x0[k * P0:(k + 1) * P0],
            x1[k * P1:(k + 1) * P1],
            x2[k * P2:(k + 1) * P2],
        ], axis=0)  # [1792, 768] f32
        xh = xc.astype(E4M3)
        xl = (xc - xh.astype(np.float32)).astype(E4M3)
        # per job: [hi 768 | lo 768], each k-major pre-transposed
        xin = np.ascontiguousarray(
            np.stack([_pretranspose(xh), _pretranspose(xl)], axis=2).reshape(
                128, NJOBS * 2 * D))
        in_maps.append(dict(xin=xin, wh=wh_np, wl=wl_np))
    return in_maps


def kernel(image, coords0, coords1, coords2, base_w, base_b,
           agg_w1, agg_b1, agg_w2, agg_b2, mlp_w1, mlp_b1, mlp_w2, mlp_b2):
    from concourse.bass_utils import run_bass_kernel_spmd

    image = np.asarray(image, dtype=np.float32)
    base_w = np.asarray(base_w, dtype=np.float32)
    base_b = np.asarray(base_b, dtype=np.float32)

    nc = _get_compiled()
    in_maps = build_in_maps(image, coords0, coords1, coords2, base_w, base_b)

    res = run_bass_kernel_spmd(nc, in_maps, core_ids=list(range(NCORES)))
    outs = [np.asarray(res.results[k]["out"], dtype=np.float32) * (1.0 / WS)
            for k in range(NCORES)]

    e0 = np.concatenate([o[0:P0] for o in outs], axis=0) + base_b
    e1 = np.concatenate([o[P0:P0 + P1] for o in outs], axis=0) + base_b
    e2 = np.concatenate([o[P0 + P1:] for o in outs], axis=0) + base_b

    if np.any(mlp_w1) or np.any(mlp_b1):
        e1 = e1 + _mlp_correction(image, np.asarray(coords1), 2,
                                  np.asarray(agg_w1, np.float32), np.asarray(agg_b1, np.float32),
                                  np.asarray(mlp_w1, np.float32), np.asarray(mlp_b1, np.float32),
                                  base_w, base_b)
    if np.any(mlp_w2) or np.any(mlp_b2):
        e2 = e2 + _mlp_correction(image, np.asarray(coords2), 4,
                                  np.asarray(agg_w2, np.float32), np.asarray(agg_b2, np.float32),
                                  np.asarray(mlp_w2, np.float32), np.asarray(mlp_b2, np.float32),
                                  base_w, base_b)

    return np.concatenate([e0, e1, e2], axis=0)


# revision 14
# speedup vs baseline: 1.2975x; 1.2975x over previous
"""Trainium2 Bass kernel for nn_AdaptivePatchEmbedding.

Reference computes, over a [3,1024,1024] image:
  e0: 16x16 patches -> flatten -> @ Wb + b                    (8192 patches)
  e1: 32x32 patches -> bilinear-resize to 16x16 -> @ Wb + b   (4096 patches)
  e2: 64x64 patches -> bilinear-resize to 16x16 -> @ Wb + b   (2048 patches)
plus a ControlNet zero-init MLP branch on e1/e2 that is exactly zero for the
zero mlp weights (host numpy fallback keeps correctness otherwise).

Identities used:
  - 16x16/stride-16 conv == flatten + matmul with Wb = base_w.reshape(D,-1).T
  - bilinear 32->16 (half-pixel) == mean of each 2x2 block
  - bilinear 64->16 == mean of the 2x2 block at rows {4i+1,4i+2} x cols {4j+1,4j+2}

Data-parallel over patches across 8 cores (the sharding hint's "gathered
patch batches"): the host gathers + pre-averages all patches (0.25 pre-scale
for e1/e2, column order (c,r,j) = Wb row order) AND pre-transposes them into
k-major layout, so the device never transposes anything:

  xin[k', g*768 + kt*128 + p] = X_job_g[p, kt*128 + k']   (bf16)

Per core: 14 jobs of 128 patches:
  direct DMA X^T k-tiles [128,768]bf16 in (job 0 on the Sync/HWDGE queue,
  jobs 1+ on the GpSimd/SWDGE queue, both racing the weight stream)
  -> 12 accumulating matmuls vs resident Wb (X^T tiles stationary, bf16,
     f32 PSUM, 448/320 bank split) -> Scalar/DVE copy-convert to bf16
  -> two half-row DMAs out (Sync queue).
Weights DMA'd in 12 chunks in exact matmul-consumption order on the
Scalar queue (the first jobs are W-stream-bound; fine chunks minimize
the stall quanta).  A short PE warm-up (plain matmuls on a memset tile,
no DMA dependency) ramps the PE clock gate while the first input DMA is
in flight.  Host adds base_b and upcasts the bf16 result.
"""

import os
import sys

for _p in ("/opt/trn_rl_repo", "/root/.axon_site/_ro/trn_rl_repo"):
    if os.path.isdir(_p) and _p not in sys.path:
        sys.path.insert(0, _p)

import numpy as np
import ml_dtypes

BF16 = ml_dtypes.bfloat16

C = 3
H = W = 1024
D = 768
BASE = 16
N0, N1, N2 = 8192, 4096, 2048
NCORES = 8
P0, P1, P2 = N0 // NCORES, N1 // NCORES, N2 // NCORES  # 1024, 512, 256
G0, G1, G2 = P0 // 128, P1 // 128, P2 // 128  # 8, 4, 2 jobs of 128 patches
NJOBS = G0 + G1 + G2  # 14
NKT = 6
NWARM = 22

_COMPILED = None


def _gather_host(image, coords0, coords1, coords2):
    """Gather + pre-average all patches into [N, 768] bf16 (per level).

    Column order is (c, r, j) = Wb's row order; e1/e2 values are the 2x2
    block means (pre-scaled by 0.25) so every row embeds with the same Wb.
    """
    imgT = np.ascontiguousarray(image.transpose(1, 2, 0))  # [H, W, C] f32
    r16 = np.arange(16)
    j16 = np.arange(16)

    y, x = coords0[:, 0], coords0[:, 1]
    m = imgT[y[:, None, None] + r16[None, :, None],
             x[:, None, None] + j16[None, None, :]]  # [N, r, j, c]
    x0 = m.transpose(0, 3, 1, 2).reshape(-1, D)

    # row-pair sums [1023, 1024, 3]
    e1r = imgT[:-1] + imgT[1:]
    # col-pair sums at the two x-phases -> [2, 1023, 512, 3]
    e1rc = np.zeros((2, H - 1, W // 2, C), np.float32)
    e1rc[0] = e1r[:, 0::2] + e1r[:, 1::2]
    e1rc[1, :, :511] = e1r[:, 1:-1:2] + e1r[:, 2::2]

    y, x = coords1[:, 0], coords1[:, 1]
    p = x & 1
    x2 = (x - p) >> 1
    m = e1rc[p[:, None, None],
             y[:, None, None] + 2 * r16[None, :, None],
             x2[:, None, None] + j16[None, None, :]]  # [N, r, j, c]
    x1 = 0.25 * m.transpose(0, 3, 1, 2).reshape(-1, D)

    y, x = coords2[:, 0], coords2[:, 1]
    p = (x + 1) & 1
    x2 = (x + 1 - p) >> 1
    m = e1rc[p[:, None, None],
             (y + 1)[:, None, None] + 4 * r16[None, :, None],
             x2[:, None, None] + 2 * j16[None, None, :]]
    x2g = 0.25 * m.transpose(0, 3, 1, 2).reshape(-1, D)

    return x0.astype(BF16), x1.astype(BF16), x2g.astype(BF16)


def _build_graph():
    import concourse.bass as bass
    import concourse.mybir as mybir
    from concourse import bacc
    import concourse.tile as tile

    nc = bacc.Bacc("TRN2", target_bir_lowering=False, debug=False)
    f32 = mybir.dt.float32
    bf16 = mybir.dt.bfloat16

    xin_d = nc.dram_tensor("xin", [128, NJOBS * D], bf16, kind="ExternalInput")
    w_d = nc.dram_tensor("wt", [128, NKT * D], bf16, kind="ExternalInput")
    out_d = nc.dram_tensor("out", [NJOBS * 128, D], bf16, kind="ExternalOutput")

    with tile.TileContext(nc) as tc:
        with (
            tc.tile_pool(name="static", bufs=1) as st,
            tc.tile_pool(name="xp", bufs=6) as xp,
            tc.tile_pool(name="psA", bufs=3, space="PSUM") as psA,
            tc.tile_pool(name="psW", bufs=2, space="PSUM") as psW,
            tc.tile_pool(name="outp", bufs=6) as outp,
        ):
            def in_job(g, eng=None):
                xt = xp.tile([128, NKT * 128], bf16, tag="xt")
                (eng or nc.gpsimd).dma_start(xt[:], xin_d[:, g * D:(g + 1) * D])
                return xt

            # job 0 rides the HWDGE path (Sync) in parallel with the Pool
            # SWDGE stream that feeds jobs 1+
            pre = [in_job(0, nc.sync)] + [in_job(g) for g in range(1, 4)]

            # weight chunks next (Scalar queue), in matmul consumption order:
            # (kt0 acc0)[0:448], (kt0 acc1)[448:768], (kt1 acc0), ... so the
            # first jobs' W-stream stalls are as fine-grained as possible
            w_t = st.tile([128, NKT * D], bf16, tag="w")
            for kt in range(NKT):
                nc.scalar.dma_start(w_t[:, kt * D:kt * D + 448],
                                    w_d[:, kt * D:kt * D + 448])
                nc.scalar.dma_start(w_t[:, kt * D + 448:(kt + 1) * D],
                                    w_d[:, kt * D + 448:(kt + 1) * D])

            ones = st.tile([128, 128], bf16, tag="ones")
            nc.vector.memset(ones[:], 0.25)
            # Scalar act-table preload off the critical path
            warm_s = st.tile([128, 8], bf16, tag="warm")
            nc.scalar.copy(warm_s[:], ones[:, 0:8])
            # PE clock-gate ramp during the first input DMA's flight
            for _ in range(NWARM):
                wps = psW.tile([128, 128], f32, tag="wp")
                nc.tensor.matmul(wps[:], ones[:], ones[:], start=True, stop=True)

            NS = 448  # Scalar-copied split; DVE takes the rest

            def embed_job(xt, g):
                acc0 = psA.tile([128, NS], f32, tag="acc0")
                acc1 = psA.tile([128, D - NS], f32, tag="acc1")
                for kt in range(NKT):
                    lhs = xt[:, kt * 128:(kt + 1) * 128]
                    nc.tensor.matmul(acc0[:], lhs, w_t[:, kt * D:kt * D + NS],
                                     start=(kt == 0), stop=(kt == NKT - 1))
                    nc.tensor.matmul(acc1[:], lhs, w_t[:, kt * D + NS:(kt + 1) * D],
                                     start=(kt == 0), stop=(kt == NKT - 1))
                o_t = outp.tile([128, D], bf16, tag="o")
                nc.scalar.copy(o_t[:, 0:NS], acc0[:])
                nc.vector.tensor_copy(o_t[:, NS:D], acc1[:])
                nc.sync.dma_start(out_d[g * 128:(g + 1) * 128, 0:NS], o_t[:, 0:NS])
                nc.sync.dma_start(out_d[g * 128:(g + 1) * 128, NS:D], o_t[:, NS:D])

            for g in range(NJOBS):
                xt = pre[g] if g < 4 else in_job(g)
                embed_job(xt[:], g)

    nc.compile()
    return nc


def _get_compiled():
    global _COMPILED
    if _COMPILED is None:
        _COMPILED = _build_graph()
    return _COMPILED


def _mlp_correction(image, coords, g, agg_w, agg_b, mlp_w, mlp_b, base_w, base_b):
    """Host fallback: the zero-init-MLP branch, exact reference math."""
    Wb = base_w.reshape(D, -1).T
    ps = BASE * g
    n = coords.shape[0]
    patches = np.empty((n, C, ps, ps), np.float32)
    for k in range(n):
        y, x = int(coords[k, 0]), int(coords[k, 1])
        patches[k] = image[:, y:y + ps, x:x + ps]
    sub = patches.reshape(n, C, g, BASE, g, BASE).transpose(0, 2, 4, 1, 3, 5)
    sub_e = sub.reshape(n, g, g, C * BASE * BASE) @ Wb + base_b
    agg = np.einsum('nhwd,odhw->no', sub_e, agg_w) + agg_b
    return agg @ mlp_w.T + mlp_b


def build_in_maps(image, coords0, coords1, coords2, base_w, base_b):
    coords0 = np.asarray(coords0).astype(np.int64)
    coords1 = np.asarray(coords1).astype(np.int64)
    coords2 = np.asarray(coords2).astype(np.int64)
    x0, x1, x2 = _gather_host(image, coords0, coords1, coords2)

    Wb = base_w.reshape(D, -1).T  # [768 k, 768 n]
    wtile = Wb.reshape(NKT, 128, D).transpose(1, 0, 2).reshape(128, NKT * D)
    wt_np = np.ascontiguousarray(wtile).astype(BF16)

    in_maps = []
    for k in range(NCORES):
        xc = np.concatenate([
            x0[k * P0:(k + 1) * P0],
            x1[k * P1:(k + 1) * P1],
            x2[k * P2:(k + 1) * P2],
        ], axis=0)  # [1792, 768]
        # pre-transpose: xin[k', g*768 + kt*128 + p] = xc[g*128+p, kt*128+k']
        xin = np.ascontiguousarray(
            xc.reshape(NJOBS, 128, NKT, 128).transpose(3, 0, 2, 1).reshape(128, NJOBS * D))
        in_maps.append(dict(xin=xin, wt=wt_np))
    return in_maps


def kernel(image, coords0, coords1, coords2, base_w, base_b,
           agg_w1, agg_b1, agg_w2, agg_b2, mlp_w1, mlp_b1, mlp_w2, mlp_b2):
    from concourse.bass_utils import run_bass_kernel_spmd

    image = np.asarray(image, dtype=np.float32)
    base_w = np.asarray(base_w, dtype=np.float32)
    base_b = np.asarray(base_b, dtype=np.float32)

    nc = _get_compiled()
    in_maps = build_in_maps(image, coords0, coords1, coords2, base_w, base_b)

    res = run_bass_kernel_spmd(nc, in_maps, core_ids=list(range(NCORES)))
    outs = [np.asarray(res.results[k]["out"], dtype=np.float32) for k in range(NCORES)]

    e0 = np.concatenate([o[0:P0] for o in outs], axis=0) + base_b
    e1 = np.concatenate([o[P0:P0 + P1] for o in outs], axis=0) + base_b
    e2 = np.concatenate([o[P0 + P1:] for o in outs], axis=0) + base_b

    if np.any(mlp_w1) or np.any(mlp_b1):
        e1 = e1 + _mlp_correction(image, np.asarray(coords1), 2,
                                  np.asarray(agg_w1, np.float32), np.asarray(agg_b1, np.float32),
                                  np.asarray(mlp_w1, np.float32), np.asarray(mlp_b1, np.float32),
                                  base_w, base_b)
    if np.any(mlp_w2) or np.any(mlp_b2):
        e2 = e2 + _mlp_correction(image, np.asarray(coords2), 4,
                                  np.asarray(agg_w2, np.float32), np.asarray(agg_b2, np.float32),
                                  np.asarray(mlp_w2, np.float32), np.asarray(mlp_b2, np.float32),
                                  base_w, base_b)

    return np.concatenate([e0, e1, e2], axis=0)
